# revision 5
# baseline (speedup 1.0000x reference)
"""Distributed Trainium2 kernel: Gemma-style attention block (B=2,T=2048,H=2048,
NH=16,NKV=4,HD=128), tensor-parallel over heads across 8 NeuronCores.

Per core c: q heads {2c, 2c+1}, kv head c//2.  Activations are kept
feature-major ("transposed", [d_part, t_free]) so every matmul contracts on the
partition dim.  Softmax is max-free (safe: rmsnorm bounds |scores| <= sqrt(HD)),
denominators and rmsnorm sum-of-squares are computed pre-broadcast via an
all-ones 128x128 stationary matmul.  o_proj partials are summed on host.

v2 engine-balance rework:
- consts split into two DMAs (phase1 weights first) to cut the startup stall
- per-block x loads coalesced 16 -> 1 DMA instruction
- phase1 rmsnorm: Square/Copy/Sqrt on ScalarE, reciprocal_approx_fast on DVE
  (replaces the 3.3us DVE reciprocal and two DVE elementwise ops)
- phase2 causal triangle: S^T/exp/den/PV restricted to valid query columns,
  single shared [128,128] upper-tri mask on the diagonal microblock only
- phase3 o_proj: PSUM->SBUF copies alternate ScalarE/VectorE
"""

import os
import sys

sys.path.insert(0, "/opt/trn_rl_repo")

import numpy as np
import ml_dtypes

import concourse.bass as bass
import concourse.mybir as mybir
import concourse.tile as tile
from concourse.bass_utils import run_bass_kernel_spmd

BF16 = ml_dtypes.bfloat16

B, T, H = 2, 2048, 2048
NH, NKV, HD = 16, 4, 128
THETA = 10000.0
EPS = 1e-6
NCORES = 8
QH = NH // NCORES          # 2 q heads per core
BT = B * T                 # 4096
NBLK = T // 512            # 4 blocks of 512 per batch
SCALE = 1.0 / np.sqrt(HD)

LAST_RESULTS = None        # stash for test harness profiling

# column offsets inside packed constants tile A [128, NCA] (phase1 needs)
OFF_WQKV = 0                      # 16*512
OFF_CQ = OFF_WQKV + 16 * 512      # 2048
OFF_CK = OFF_CQ + T               # 2048
OFF_SIN = OFF_CK + T              # 2048
OFF_RQ = OFF_SIN + T              # 128
OFF_RK = OFF_RQ + HD              # 128
OFF_ONES = OFF_RK + HD            # 128
NCA = OFF_ONES + 128

# constants tile B [128, NCB] (needed from phase2 on)
OFF_WO = 0                        # QH*2048
OFF_TRI = OFF_WO + QH * H         # 128 upper-tri (col >= row)
NCB = OFF_TRI + 128


def _rope_tables(w_q, w_k):
    """rope(w*q) = cosw ⊙ q + sin ⊙ (R_w @ q) where cosw = cos·(1+w) and
    R_w = rot_half matrix with the ±1 and the (1+w) source weight folded in.
    Returns cosw_q, cosw_k, sin (plain), rotmT_q, rotmT_k (lhsT layout)."""
    inv = 1.0 / (THETA ** (np.arange(0, HD, 2, dtype=np.float64) / HD))  # [64]
    t = np.arange(T, dtype=np.float64)
    fr = np.outer(inv, t)                      # [64, T]
    emb = np.concatenate([fr, fr], 0)          # [HD, T]
    cos, sin = np.cos(emb), np.sin(emb)
    cosws, rotms = [], []
    for w in (w_q, w_k):
        wp = 1.0 + w.astype(np.float64)
        cosws.append((cos * wp[:, None]).astype(BF16))
        R = np.zeros((HD, HD))
        for m in range(64):
            R[m, m + 64] = -wp[m + 64]
        for m in range(64, HD):
            R[m, m - 64] = +wp[m - 64]
        rotms.append(np.ascontiguousarray(R.T).astype(BF16))  # lhsT[k, m] = R[m, k]
    return cosws[0], cosws[1], sin.astype(BF16), rotms[0], rotms[1]


def _legalize_waits(nc):
    """This container's walrus accepts only ONE sync wait per instruction
    (even shipped Tile kernels fail codegen). Split each multi-wait
    instruction into single-wait NOPs on the same engine followed by the
    original holding the last wait — per-engine program order makes this
    exactly equivalent."""
    nid = 0
    for fn in nc.m.functions:
        for blk in fn.blocks:
            out = []
            for inst in blk.instructions:
                si = getattr(inst, "sync_info", None)
                if si is not None and si.on_wait and len(si.on_wait) > 1:
                    waits = list(si.on_wait)
                    ups = list(si.on_update) if si.on_update else []
                    for w in waits[:-1]:
                        nop = mybir.InstNoOp(name=f"swx-{nid}", ins=[], outs=[])
                        nid += 1
                        nop.engine = inst.engine
                        nop.sync_info = mybir.SyncInfo(on_wait=[w], on_update=[])
                        out.append(nop)
                    inst.sync_info = mybir.SyncInfo(
                        on_wait=[waits[-1]], on_update=ups)
                out.append(inst)
            blk.instructions = out
    return nc


def _build_graph(perturb=0, repeat=1, cfg=None):
    cfg = {**dict(xtp=3, tmp=6, pacc=2, pmm=4, depth=3, fuse3=0), **(cfg or {})}
    nc = bass.Bass()
    f32, bf16 = mybir.dt.float32, mybir.dt.bfloat16

    xT = nc.dram_tensor("xT", [H, BT], bf16, kind="ExternalInput")
    constsA = nc.dram_tensor("constsA", [128, NCA], bf16, kind="ExternalInput")
    constsB = nc.dram_tensor("constsB", [128, NCB], bf16, kind="ExternalInput")
    out = nc.dram_tensor("out", [BT, H], bf16, kind="ExternalOutput")

    # dram view for coalesced per-block x loads: (ht p) t -> p ht t
    xT3 = xT[:, :].rearrange("(ht p) t -> p ht t", ht=16)

    with tile.TileContext(nc) as tc:
        with (
            tc.tile_pool(name="singles", bufs=1) as singles,
            tc.tile_pool(name="xtp", bufs=cfg["xtp"]) as xtp,
            tc.tile_pool(name="tmp", bufs=cfg["tmp"]) as tmp,
            tc.tile_pool(name="psum", bufs=cfg["pacc"], space="PSUM") as pacc,
            tc.tile_pool(name="psmm", bufs=cfg["pmm"], space="PSUM") as pmm,
        ):
            # ---- resident constants: two DMAs, phase1 needs first ----
            constsA_sb = singles.tile([128, NCA], bf16)
            nc.sync.dma_start(out=constsA_sb, in_=constsA[:, :])
            constsB_sb = singles.tile([128, NCB], bf16)
            # other HWDGE ring (ACT) so it doesn't queue ahead of x loads
            nc.scalar.dma_start(out=constsB_sb, in_=constsB[:, :])
            wqkv_sb = constsA_sb[:, OFF_WQKV:OFF_WQKV + 16 * 512]
            cq_sb = constsA_sb[:, OFF_CQ:OFF_CQ + T]
            ck_sb = constsA_sb[:, OFF_CK:OFF_CK + T]
            sin_sb = constsA_sb[:, OFF_SIN:OFF_SIN + T]
            rq_sb = constsA_sb[:, OFF_RQ:OFF_RQ + HD]
            rk_sb = constsA_sb[:, OFF_RK:OFF_RK + HD]
            ones_sb = constsA_sb[:, OFF_ONES:OFF_ONES + 128]
            wo_sb = constsB_sb[:, OFF_WO:OFF_WO + QH * H]
            tri_sb = constsB_sb[:, OFF_TRI:OFF_TRI + 128]
            for _ in range(perturb):
                nc.sync.nop()

            # ---- per-batch activations (feature-major) ----
            qT = [singles.tile([128, QH * T], bf16, name=f"qT{b}", tag=f"qT{b}")
                  for b in range(B)]
            kT = [singles.tile([128, T], bf16, name=f"kT{b}", tag=f"kT{b}")
                  for b in range(B)]
            vn = [singles.tile([128, 16 * 128], bf16, name=f"vn{b}", tag=f"vn{b}")
                  for b in range(B)]
            attnT = [singles.tile([128, QH * T], bf16, name=f"attnT{b}", tag=f"attnT{b}")
                     for b in range(B)]

            ACT_F = mybir.ActivationFunctionType

            def phase1(b):
                for blk in range(NBLK):
                    t0 = blk * 512
                    bt0 = b * T + t0
                    xt_all = xtp.tile([128, 16 * 512], bf16, tag="xt")
                    nc.sync.dma_start(
                        out=xt_all.rearrange("p (ht c) -> p ht c", ht=16),
                        in_=xT3[:, :, bt0:bt0 + 512])
                    xts = [xt_all[:, ht * 512:(ht + 1) * 512] for ht in range(16)]
                    # q0, q1, k projections (feature-major out)
                    for dt in range(3):
                        ps = pacc.tile([128, 512], f32, tag="acc")
                        for ht in range(16):
                            nc.tensor.matmul(
                                ps,
                                lhsT=wqkv_sb[:, ht * 512 + dt * 128:ht * 512 + (dt + 1) * 128],
                                rhs=xts[ht], start=(ht == 0), stop=(ht == 15))
                        sq = tmp.tile([128, 512], bf16, tag="sq")
                        nc.scalar.activation(
                            out=sq, in_=ps, func=ACT_F.Square)
                        traw = tmp.tile([128, 512], bf16, tag="traw")
                        nc.scalar.activation(
                            out=traw, in_=ps, func=ACT_F.Copy)
                        ssq = pmm.tile([128, 512], f32, tag="mm")
                        nc.tensor.matmul(ssq, lhsT=ones_sb, rhs=sq, start=True, stop=True)
                        # rstd = exp(-0.5*ln(ssq) + 0.5*ln(HD)) = 1/sqrt(ssq/HD)
                        # on ScalarE (idle in phase1); DVE reciprocal is 6 cpe.
                        lssq = tmp.tile([128, 512], f32, tag="std")
                        nc.scalar.activation(
                            out=lssq, in_=ssq, func=ACT_F.Ln, scale=1.0 / HD)
                        rstd = tmp.tile([128, 512], bf16, tag="rstd")
                        nc.scalar.activation(
                            out=rstd, in_=lssq, func=ACT_F.Exp, scale=-0.5)
                        cos_t, rot_t = (cq_sb, rq_sb) if dt < 2 else (ck_sb, rk_sb)
                        t1 = tmp.tile([128, 512], bf16, tag="t1")
                        nc.vector.tensor_mul(t1, traw, cos_t[:, t0:t0 + 512])
                        rps = pmm.tile([128, 512], f32, tag="mm")
                        nc.tensor.matmul(rps, lhsT=rot_t, rhs=traw, start=True, stop=True)
                        t2 = tmp.tile([128, 512], bf16, tag="t2")
                        nc.vector.tensor_mul(t2, rps, sin_sb[:, t0:t0 + 512])
                        nc.vector.tensor_add(out=t1, in0=t1, in1=t2)
                        dest = (qT[b][:, dt * T + t0:dt * T + t0 + 512] if dt < 2
                                else kT[b][:, t0:t0 + 512])
                        nc.vector.tensor_mul(dest, t1, rstd)
                    # v projection, natural layout [t_part, d_free]
                    vps = pacc.tile([128, 512], f32, tag="acc")
                    for c4 in range(4):
                        for ht in range(16):
                            nc.tensor.matmul(
                                vps[:, c4 * 128:(c4 + 1) * 128],
                                lhsT=xts[ht][:, c4 * 128:(c4 + 1) * 128],
                                rhs=wqkv_sb[:, ht * 512 + 384:ht * 512 + 512],
                                start=(ht == 0), stop=(ht == 15))
                    with nc.allow_low_precision(reason="bf16 act copy"):
                        nc.vector.tensor_copy(
                            out=vn[b][:, blk * 512:(blk + 1) * 512], in_=vps)

            def attn_block(b, h, j):
                # Software-pipelined: S^T matmuls issued DEPTH tiles ahead so
                # the PE never stalls on the ACT exp of the current tile.
                # Causal restriction: tile i >= 4j only covers query columns
                # >= 128*(i-4j); the 128-wide boundary microblock gets the
                # shared upper-tri mask.
                DEPTH = cfg["depth"]
                ntk = 4 * j + 4
                aps = pacc.tile([128, 512], f32, tag="acc")
                dps = pacc.tile([128, 512], f32, tag="den")
                sps_l, pt_l, c0_l = [], [], []

                def issue_st(i):
                    r = i - 4 * j
                    c0 = 128 * r if r > 0 else 0
                    sps = pmm.tile([128, 512], f32, tag="mm", name="sps")
                    nc.tensor.matmul(
                        sps[:, c0:], lhsT=kT[b][:, i * 128:(i + 1) * 128],
                        rhs=qT[b][:, h * T + j * 512 + c0:h * T + (j + 1) * 512],
                        start=True, stop=True)
                    sps_l.append(sps)
                    c0_l.append(c0)

                def issue_exp(i):
                    r = i - 4 * j
                    c0 = c0_l[i]
                    pt = tmp.tile([128, 512], bf16, tag="pt", name="pt")
                    nc.scalar.activation(
                        out=pt[:, c0:], in_=sps_l[i][:, c0:],
                        func=ACT_F.Exp, scale=SCALE)
                    if r >= 0:
                        nc.vector.tensor_mul(
                            pt[:, c0:c0 + 128], pt[:, c0:c0 + 128], tri_sb)
                    pt_l.append(pt)

                for i in range(min(DEPTH, ntk)):
                    issue_st(i)
                issue_exp(0)
                for i in range(ntk):
                    if i + DEPTH < ntk:
                        issue_st(i + DEPTH)
                    if i + 1 < ntk:
                        issue_exp(i + 1)
                    c0 = c0_l[i]
                    nc.tensor.matmul(dps[:, c0:], lhsT=ones_sb, rhs=pt_l[i][:, c0:],
                                     start=(i == 0), stop=(i == ntk - 1))
                    nc.tensor.matmul(aps[:, c0:], lhsT=vn[b][:, i * 128:(i + 1) * 128],
                                     rhs=pt_l[i][:, c0:], start=(i == 0),
                                     stop=(i == ntk - 1))
                recip = tmp.tile([128, 512], mybir.dt.float32, tag="rec")
                nc.vector.reciprocal(out=recip, in_=dps)
                nc.vector.tensor_mul(
                    attnT[b][:, h * T + j * 512:h * T + (j + 1) * 512], aps, recip)

            def phase2(b):
                for h in range(QH):
                    for j in range(NBLK):
                        attn_block(b, h, j)

            def oproj_tile(b, m, j, neng):
                ops = pmm.tile([128, 512], f32, tag="mm", name="ops")
                for hh in range(QH):
                    nc.tensor.matmul(
                        ops,
                        lhsT=attnT[b][:, hh * T + m * 128:hh * T + (m + 1) * 128],
                        rhs=wo_sb[:, hh * H + j * 512:hh * H + (j + 1) * 512],
                        start=(hh == 0), stop=(hh == QH - 1))
                osb = tmp.tile([128, 512], bf16, tag="osb", name="osb")
                if neng % 2 == 0:
                    with nc.allow_low_precision(reason="bf16 partials, host-summed f32"):
                        nc.vector.tensor_copy(out=osb, in_=ops)
                else:
                    nc.scalar.activation(out=osb, in_=ops, func=ACT_F.Copy)
                nc.sync.dma_start(
                    out=out[b * T + m * 128:b * T + (m + 1) * 128,
                            j * 512:(j + 1) * 512],
                    in_=osb)

            def phase3(b):
                n = 0
                for m in range(16):
                    for j in range(NBLK):
                        oproj_tile(b, m, j, n)
                        n += 1

            def phase23_fused(b):
                n = 0
                for j in range(NBLK):
                    for h in range(QH):
                        attn_block(b, h, j)
                    for m in range(4 * j, 4 * j + 4):
                        for jo in range(NBLK):
                            oproj_tile(b, m, jo, n)
                            n += 1

            for _ in range(repeat):   # >1 only for benchmarking (idempotent)
                if cfg["fuse3"]:
                    phase1(0)
                    phase23_fused(0)
                    phase1(1)
                    phase23_fused(1)
                else:
                    phase1(0)
                    phase2(0)
                    phase1(1)
                    phase3(0)
                    phase2(1)
                    phase3(1)
    return nc


_GRAPH = None


def kernel(x, Wq, Wk, Wv, Wo, q_norm_w, k_norm_w):
    global _GRAPH, LAST_RESULTS
    x = np.asarray(x, dtype=np.float32)
    Wq = np.asarray(Wq, dtype=np.float32)
    Wk = np.asarray(Wk, dtype=np.float32)
    Wv = np.asarray(Wv, dtype=np.float32)
    Wo = np.asarray(Wo, dtype=np.float32)
    q_norm_w = np.asarray(q_norm_w, dtype=np.float32)
    k_norm_w = np.asarray(k_norm_w, dtype=np.float32)

    xT = np.ascontiguousarray(x.reshape(BT, H).T).astype(BF16)
    cos_q, cos_k, sin_d, rotm_q, rotm_k = _rope_tables(q_norm_w, k_norm_w)
    p = np.arange(128)[:, None]
    f = np.arange(128)[None, :]
    tri = (f >= p).astype(BF16)       # upper-tri incl diagonal

    in_maps = []
    for c in range(NCORES):
        kv = c // 2
        w_all = np.concatenate([
            Wq[QH * HD * c:QH * HD * (c + 1)],
            Wk[HD * kv:HD * (kv + 1)],
            Wv[HD * kv:HD * (kv + 1)]], 0)              # [512, H]
        wqkvT = np.ascontiguousarray(w_all.T).astype(BF16)       # [H, 512]
        woT = np.ascontiguousarray(
            Wo[:, QH * HD * c:QH * HD * (c + 1)].T).astype(BF16)  # [QH*HD, H]
        ca = np.zeros((128, NCA), dtype=BF16)
        ca[:, OFF_WQKV:OFF_WQKV + 16 * 512] = (
            wqkvT.reshape(16, 128, 512).transpose(1, 0, 2).reshape(128, 16 * 512))
        ca[:, OFF_CQ:OFF_CQ + T] = cos_q
        ca[:, OFF_CK:OFF_CK + T] = cos_k
        ca[:, OFF_SIN:OFF_SIN + T] = sin_d
        ca[:, OFF_RQ:OFF_RQ + HD] = rotm_q
        ca[:, OFF_RK:OFF_RK + HD] = rotm_k
        ca[:, OFF_ONES:OFF_ONES + 128] = 1.0
        cb = np.zeros((128, NCB), dtype=BF16)
        cb[:, OFF_WO:OFF_WO + QH * H] = (
            woT.reshape(QH, 128, H).transpose(1, 0, 2).reshape(128, QH * H))
        cb[:, OFF_TRI:OFF_TRI + 128] = tri
        in_maps.append({"xT": xT, "constsA": ca, "constsB": cb})

    if _GRAPH is None:
        _GRAPH = _legalize_waits(_build_graph())

    want_trace = bool(int(os.environ.get("ATTN_TRACE", "0")))
    try:
        res = run_bass_kernel_spmd(
            _GRAPH, in_maps, core_ids=list(range(NCORES)), trace=want_trace)
    except ModuleNotFoundError:
        if not want_trace:
            raise
        # axon NTFF profile hook unavailable in this environment
        res = run_bass_kernel_spmd(
            _GRAPH, in_maps, core_ids=list(range(NCORES)), trace=False)
    LAST_RESULTS = res
    acc = np.zeros((BT, H), dtype=np.float32)
    for r in res.results:
        acc += r["out"]
    return acc.reshape(B, T, H)


# revision 12
# speedup vs baseline: 1.1011x; 1.1011x over previous
"""Distributed Trainium2 kernel: Gemma-style attention block (B=2,T=2048,H=2048,
NH=16,NKV=4,HD=128), tensor-parallel over heads across 8 NeuronCores.

Per core c: q heads {2c, 2c+1}, kv head c//2.  Activations are kept
feature-major ("transposed", [d_part, t_free]) so every matmul contracts on the
partition dim.  Softmax is max-free (safe: rmsnorm bounds |scores| <= sqrt(HD)),
denominators and rmsnorm sum-of-squares are computed pre-broadcast via an
all-ones 128x128 stationary matmul.  o_proj partials are summed on host.

v2 engine-balance rework:
- consts split into two DMAs (phase1 weights first) to cut the startup stall
- per-block x loads coalesced 16 -> 1 DMA instruction
- phase1 rmsnorm: Square/Copy/Sqrt on ScalarE, reciprocal_approx_fast on DVE
  (replaces the 3.3us DVE reciprocal and two DVE elementwise ops)
- phase2 causal triangle: S^T/exp/den/PV restricted to valid query columns,
  single shared [128,128] upper-tri mask on the diagonal microblock only
- phase3 o_proj: PSUM->SBUF copies alternate ScalarE/VectorE
"""

import os
import sys

sys.path.insert(0, "/opt/trn_rl_repo")

import numpy as np
import ml_dtypes

import concourse.bass as bass
import concourse.mybir as mybir
import concourse.tile as tile
from concourse.bass_utils import run_bass_kernel_spmd

BF16 = ml_dtypes.bfloat16

B, T, H = 2, 2048, 2048
NH, NKV, HD = 16, 4, 128
THETA = 10000.0
EPS = 1e-6
NCORES = 8
QH = NH // NCORES          # 2 q heads per core
BT = B * T                 # 4096
NBLK = T // 512            # 4 blocks of 512 per batch
SCALE = 1.0 / np.sqrt(HD)

LAST_RESULTS = None        # stash for test harness profiling

# column offsets inside packed constants tile A [128, NCA] (phase1 needs)
OFF_WQKV = 0                      # 16*512
OFF_CQ = OFF_WQKV + 16 * 512      # 2048
OFF_CK = OFF_CQ + T               # 2048
OFF_SIN = OFF_CK + T              # 2048
OFF_RQ = OFF_SIN + T              # 128
OFF_RK = OFF_RQ + HD              # 128
OFF_ONES = OFF_RK + HD            # 128
NCA = OFF_ONES + 128

# constants tile B [128, NCB] (needed from phase2 on)
OFF_WO = 0                        # QH*2048
OFF_TRI = OFF_WO + QH * H         # 128 upper-tri (col >= row)
NCB = OFF_TRI + 128


def _rope_tables(w_q, w_k):
    """rope(w*q) = cosw ⊙ q + sin ⊙ (R_w @ q) where cosw = cos·(1+w) and
    R_w = rot_half matrix with the ±1 and the (1+w) source weight folded in.
    Returns cosw_q, cosw_k, sin (plain), rotmT_q, rotmT_k (lhsT layout)."""
    inv = 1.0 / (THETA ** (np.arange(0, HD, 2, dtype=np.float64) / HD))  # [64]
    t = np.arange(T, dtype=np.float64)
    fr = np.outer(inv, t)                      # [64, T]
    emb = np.concatenate([fr, fr], 0)          # [HD, T]
    cos, sin = np.cos(emb), np.sin(emb)
    cosws, rotms = [], []
    for w in (w_q, w_k):
        wp = 1.0 + w.astype(np.float64)
        cosws.append((cos * wp[:, None]).astype(BF16))
        R = np.zeros((HD, HD))
        for m in range(64):
            R[m, m + 64] = -wp[m + 64]
        for m in range(64, HD):
            R[m, m - 64] = +wp[m - 64]
        rotms.append(np.ascontiguousarray(R.T).astype(BF16))  # lhsT[k, m] = R[m, k]
    return cosws[0], cosws[1], sin.astype(BF16), rotms[0], rotms[1]


def _legalize_waits(nc):
    """This container's walrus accepts only ONE sync wait per instruction
    (even shipped Tile kernels fail codegen). Split each multi-wait
    instruction into single-wait NOPs on the same engine followed by the
    original holding the last wait — per-engine program order makes this
    exactly equivalent."""
    nid = 0
    for fn in nc.m.functions:
        for blk in fn.blocks:
            out = []
            for inst in blk.instructions:
                si = getattr(inst, "sync_info", None)
                if si is not None and si.on_wait and len(si.on_wait) > 1:
                    waits = list(si.on_wait)
                    ups = list(si.on_update) if si.on_update else []
                    for w in waits[:-1]:
                        nop = mybir.InstNoOp(name=f"swx-{nid}", ins=[], outs=[])
                        nid += 1
                        nop.engine = inst.engine
                        nop.sync_info = mybir.SyncInfo(on_wait=[w], on_update=[])
                        out.append(nop)
                    inst.sync_info = mybir.SyncInfo(
                        on_wait=[waits[-1]], on_update=ups)
                out.append(inst)
            blk.instructions = out
    return nc


def _build_graph(perturb=0, repeat=1, cfg=None):
    cfg = {**dict(xtp=3, tmp=6, pacc=2, pmm=4, depth=3, fuse3=0), **(cfg or {})}
    nc = bass.Bass()
    f32, bf16 = mybir.dt.float32, mybir.dt.bfloat16

    # x pre-tiled on host: row bi*128+p, col ht*512+c = x^T[ht*128+p, block bi col c]
    xB = nc.dram_tensor("xB", [B * NBLK * 128, 16 * 512], bf16, kind="ExternalInput")
    constsA = nc.dram_tensor("constsA", [128, NCA], bf16, kind="ExternalInput")
    constsB = nc.dram_tensor("constsB", [128, NCB], bf16, kind="ExternalInput")
    out = nc.dram_tensor("out", [BT, H], bf16, kind="ExternalOutput")

    with tile.TileContext(nc) as tc:
        with (
            tc.tile_pool(name="singles", bufs=1) as singles,
            tc.tile_pool(name="xtp", bufs=cfg["xtp"]) as xtp,
            tc.tile_pool(name="tmp", bufs=cfg["tmp"]) as tmp,
            tc.tile_pool(name="psum", bufs=cfg["pacc"], space="PSUM") as pacc,
            tc.tile_pool(name="psmm", bufs=cfg["pmm"], space="PSUM") as pmm,
        ):
            # ---- resident constants: two DMAs, phase1 needs first ----
            # constsA on the ACT HWDGE ring so it runs in parallel with the
            # first x block load on the SP ring; constsB afterwards.
            constsA_sb = singles.tile([128, NCA], bf16)
            nc.scalar.dma_start(out=constsA_sb, in_=constsA[:, :])
            constsB_sb = singles.tile([128, NCB], bf16)
            nc.scalar.dma_start(out=constsB_sb, in_=constsB[:, :])
            wqkv_sb = constsA_sb[:, OFF_WQKV:OFF_WQKV + 16 * 512]
            cq_sb = constsA_sb[:, OFF_CQ:OFF_CQ + T]
            ck_sb = constsA_sb[:, OFF_CK:OFF_CK + T]
            sin_sb = constsA_sb[:, OFF_SIN:OFF_SIN + T]
            rq_sb = constsA_sb[:, OFF_RQ:OFF_RQ + HD]
            rk_sb = constsA_sb[:, OFF_RK:OFF_RK + HD]
            ones_sb = constsA_sb[:, OFF_ONES:OFF_ONES + 128]
            wo_sb = constsB_sb[:, OFF_WO:OFF_WO + QH * H]
            tri_sb = constsB_sb[:, OFF_TRI:OFF_TRI + 128]
            for _ in range(perturb):
                nc.sync.nop()

            # ---- per-batch activations (feature-major) ----
            qT = [singles.tile([128, QH * T], bf16, name=f"qT{b}", tag=f"qT{b}")
                  for b in range(B)]
            kT = [singles.tile([128, T], bf16, name=f"kT{b}", tag=f"kT{b}")
                  for b in range(B)]
            vn = [singles.tile([128, 16 * 128], bf16, name=f"vn{b}", tag=f"vn{b}")
                  for b in range(B)]
            attnT = [singles.tile([128, QH * T], bf16, name=f"attnT{b}", tag=f"attnT{b}")
                     for b in range(B)]

            ACT_F = mybir.ActivationFunctionType

            def phase1(b):
                for blk in range(NBLK):
                    t0 = blk * 512
                    bt0 = b * T + t0
                    bi = b * NBLK + blk
                    xt_all = xtp.tile([128, 16 * 512], bf16, tag="xt")
                    nc.sync.dma_start(
                        out=xt_all, in_=xB[bi * 128:(bi + 1) * 128, :])
                    xts = [xt_all[:, ht * 512:(ht + 1) * 512] for ht in range(16)]
                    # q0, q1, k projections (feature-major out)
                    for dt in range(3):
                        ps = pacc.tile([128, 512], f32, tag="acc")
                        for ht in range(16):
                            nc.tensor.matmul(
                                ps,
                                lhsT=wqkv_sb[:, ht * 512 + dt * 128:ht * 512 + (dt + 1) * 128],
                                rhs=xts[ht], start=(ht == 0), stop=(ht == 15))
                        sq = tmp.tile([128, 512], bf16, tag="sq")
                        nc.scalar.activation(
                            out=sq, in_=ps, func=ACT_F.Square)
                        traw = tmp.tile([128, 512], bf16, tag="traw")
                        nc.scalar.activation(
                            out=traw, in_=ps, func=ACT_F.Copy)
                        ssq = pmm.tile([128, 512], f32, tag="mm")
                        nc.tensor.matmul(ssq, lhsT=ones_sb, rhs=sq, start=True, stop=True)
                        # rstd = exp(-0.5*ln(ssq) + 0.5*ln(HD)) = 1/sqrt(ssq/HD)
                        # on ScalarE (idle in phase1); DVE reciprocal is 6 cpe.
                        lssq = tmp.tile([128, 512], f32, tag="std")
                        nc.scalar.activation(
                            out=lssq, in_=ssq, func=ACT_F.Ln, scale=1.0 / HD)
                        rstd = tmp.tile([128, 512], bf16, tag="rstd")
                        nc.scalar.activation(
                            out=rstd, in_=lssq, func=ACT_F.Exp, scale=-0.5)
                        cos_t, rot_t = (cq_sb, rq_sb) if dt < 2 else (ck_sb, rk_sb)
                        t1 = tmp.tile([128, 512], bf16, tag="t1")
                        nc.vector.tensor_mul(t1, traw, cos_t[:, t0:t0 + 512])
                        rps = pmm.tile([128, 512], f32, tag="mm")
                        nc.tensor.matmul(rps, lhsT=rot_t, rhs=traw, start=True, stop=True)
                        t2 = tmp.tile([128, 512], bf16, tag="t2")
                        nc.vector.tensor_mul(t2, rps, sin_sb[:, t0:t0 + 512])
                        nc.vector.tensor_add(out=t1, in0=t1, in1=t2)
                        dest = (qT[b][:, dt * T + t0:dt * T + t0 + 512] if dt < 2
                                else kT[b][:, t0:t0 + 512])
                        nc.vector.tensor_mul(dest, t1, rstd)
                    # v projection, natural layout [t_part, d_free]
                    vps = pacc.tile([128, 512], f32, tag="acc")
                    for c4 in range(4):
                        for ht in range(16):
                            nc.tensor.matmul(
                                vps[:, c4 * 128:(c4 + 1) * 128],
                                lhsT=xts[ht][:, c4 * 128:(c4 + 1) * 128],
                                rhs=wqkv_sb[:, ht * 512 + 384:ht * 512 + 512],
                                start=(ht == 0), stop=(ht == 15))
                    with nc.allow_low_precision(reason="bf16 act copy"):
                        nc.vector.tensor_copy(
                            out=vn[b][:, blk * 512:(blk + 1) * 512], in_=vps)

            def attn_block(b, h, j):
                # Software-pipelined: S^T matmuls issued DEPTH tiles ahead so
                # the PE never stalls on the ACT exp of the current tile.
                # Causal restriction: tile i >= 4j only covers query columns
                # >= 128*(i-4j); the 128-wide boundary microblock gets the
                # shared upper-tri mask.
                DEPTH = cfg["depth"]
                ntk = 4 * j + 4
                aps = pacc.tile([128, 512], f32, tag="acc")
                dps = pacc.tile([128, 512], f32, tag="den")
                sps_l, pt_l, c0_l = [], [], []

                def issue_st(i):
                    r = i - 4 * j
                    c0 = 128 * r if r > 0 else 0
                    sps = pmm.tile([128, 512], f32, tag="mm", name="sps")
                    nc.tensor.matmul(
                        sps[:, c0:], lhsT=kT[b][:, i * 128:(i + 1) * 128],
                        rhs=qT[b][:, h * T + j * 512 + c0:h * T + (j + 1) * 512],
                        start=True, stop=True)
                    sps_l.append(sps)
                    c0_l.append(c0)

                def issue_exp(i):
                    r = i - 4 * j
                    c0 = c0_l[i]
                    pt = tmp.tile([128, 512], bf16, tag="pt", name="pt")
                    nc.scalar.activation(
                        out=pt[:, c0:], in_=sps_l[i][:, c0:],
                        func=ACT_F.Exp, scale=SCALE)
                    if r >= 0:
                        nc.vector.tensor_mul(
                            pt[:, c0:c0 + 128], pt[:, c0:c0 + 128], tri_sb)
                    pt_l.append(pt)

                for i in range(min(DEPTH, ntk)):
                    issue_st(i)
                issue_exp(0)
                for i in range(ntk):
                    if i + DEPTH < ntk:
                        issue_st(i + DEPTH)
                    if i + 1 < ntk:
                        issue_exp(i + 1)
                    c0 = c0_l[i]
                    nc.tensor.matmul(dps[:, c0:], lhsT=ones_sb, rhs=pt_l[i][:, c0:],
                                     start=(i == 0), stop=(i == ntk - 1))
                    nc.tensor.matmul(aps[:, c0:], lhsT=vn[b][:, i * 128:(i + 1) * 128],
                                     rhs=pt_l[i][:, c0:], start=(i == 0),
                                     stop=(i == ntk - 1))
                recip = tmp.tile([128, 512], mybir.dt.float32, tag="rec")
                nc.vector.reciprocal(out=recip, in_=dps)
                nc.vector.tensor_mul(
                    attnT[b][:, h * T + j * 512:h * T + (j + 1) * 512], aps, recip)

            def phase2(b):
                for h in range(QH):
                    for j in range(NBLK):
                        attn_block(b, h, j)

            def oproj_tile(b, m, j, neng):
                ops = pmm.tile([128, 512], f32, tag="mm", name="ops")
                for hh in range(QH):
                    nc.tensor.matmul(
                        ops,
                        lhsT=attnT[b][:, hh * T + m * 128:hh * T + (m + 1) * 128],
                        rhs=wo_sb[:, hh * H + j * 512:hh * H + (j + 1) * 512],
                        start=(hh == 0), stop=(hh == QH - 1))
                osb = tmp.tile([128, 512], bf16, tag="osb", name="osb")
                if neng % 4 < 3:
                    with nc.allow_low_precision(reason="bf16 partials, host-summed f32"):
                        nc.vector.tensor_copy(out=osb, in_=ops)
                else:
                    nc.scalar.activation(out=osb, in_=ops, func=ACT_F.Copy)
                nc.sync.dma_start(
                    out=out[b * T + m * 128:b * T + (m + 1) * 128,
                            j * 512:(j + 1) * 512],
                    in_=osb)

            def phase3(b):
                n = 0
                for m in range(16):
                    for j in range(NBLK):
                        oproj_tile(b, m, j, n)
                        n += 1

            def phase23_fused(b):
                n = 0
                for j in range(NBLK):
                    for h in range(QH):
                        attn_block(b, h, j)
                    for m in range(4 * j, 4 * j + 4):
                        for jo in range(NBLK):
                            oproj_tile(b, m, jo, n)
                            n += 1

            for _ in range(repeat):   # >1 only for benchmarking (idempotent)
                if cfg["fuse3"]:
                    phase1(0)
                    phase23_fused(0)
                    phase1(1)
                    phase23_fused(1)
                else:
                    phase1(0)
                    phase2(0)
                    phase1(1)
                    phase3(0)
                    phase2(1)
                    phase3(1)
    return nc


_GRAPH = None


def kernel(x, Wq, Wk, Wv, Wo, q_norm_w, k_norm_w):
    global _GRAPH, LAST_RESULTS
    x = np.asarray(x, dtype=np.float32)
    Wq = np.asarray(Wq, dtype=np.float32)
    Wk = np.asarray(Wk, dtype=np.float32)
    Wv = np.asarray(Wv, dtype=np.float32)
    Wo = np.asarray(Wo, dtype=np.float32)
    q_norm_w = np.asarray(q_norm_w, dtype=np.float32)
    k_norm_w = np.asarray(k_norm_w, dtype=np.float32)

    xT = np.ascontiguousarray(x.reshape(BT, H).T).astype(BF16)
    # pre-tiled blocks: xB[bi*128+p, ht*512+c] = xT[ht*128+p, bi*512+c]
    xBm = np.ascontiguousarray(
        xT.reshape(16, 128, B * NBLK, 512).transpose(2, 1, 0, 3)
        .reshape(B * NBLK * 128, 16 * 512))
    cos_q, cos_k, sin_d, rotm_q, rotm_k = _rope_tables(q_norm_w, k_norm_w)
    p = np.arange(128)[:, None]
    f = np.arange(128)[None, :]
    tri = (f >= p).astype(BF16)       # upper-tri incl diagonal

    in_maps = []
    for c in range(NCORES):
        kv = c // 2
        w_all = np.concatenate([
            Wq[QH * HD * c:QH * HD * (c + 1)],
            Wk[HD * kv:HD * (kv + 1)],
            Wv[HD * kv:HD * (kv + 1)]], 0)              # [512, H]
        wqkvT = np.ascontiguousarray(w_all.T).astype(BF16)       # [H, 512]
        woT = np.ascontiguousarray(
            Wo[:, QH * HD * c:QH * HD * (c + 1)].T).astype(BF16)  # [QH*HD, H]
        ca = np.zeros((128, NCA), dtype=BF16)
        ca[:, OFF_WQKV:OFF_WQKV + 16 * 512] = (
            wqkvT.reshape(16, 128, 512).transpose(1, 0, 2).reshape(128, 16 * 512))
        ca[:, OFF_CQ:OFF_CQ + T] = cos_q
        ca[:, OFF_CK:OFF_CK + T] = cos_k
        ca[:, OFF_SIN:OFF_SIN + T] = sin_d
        ca[:, OFF_RQ:OFF_RQ + HD] = rotm_q
        ca[:, OFF_RK:OFF_RK + HD] = rotm_k
        ca[:, OFF_ONES:OFF_ONES + 128] = 1.0
        cb = np.zeros((128, NCB), dtype=BF16)
        cb[:, OFF_WO:OFF_WO + QH * H] = (
            woT.reshape(QH, 128, H).transpose(1, 0, 2).reshape(128, QH * H))
        cb[:, OFF_TRI:OFF_TRI + 128] = tri
        in_maps.append({"xB": xBm, "constsA": ca, "constsB": cb})

    if _GRAPH is None:
        import json as _json
        cfg = _json.loads(os.environ.get("ATTN_CFG", "{}")) or None
        _GRAPH = _legalize_waits(_build_graph(cfg=cfg))

    want_trace = bool(int(os.environ.get("ATTN_TRACE", "0")))
    try:
        res = run_bass_kernel_spmd(
            _GRAPH, in_maps, core_ids=list(range(NCORES)), trace=want_trace)
    except ModuleNotFoundError:
        if not want_trace:
            raise
        # axon NTFF profile hook unavailable in this environment
        res = run_bass_kernel_spmd(
            _GRAPH, in_maps, core_ids=list(range(NCORES)), trace=False)
    LAST_RESULTS = res
    acc = np.zeros((BT, H), dtype=np.float32)
    for r in res.results:
        acc += r["out"]
    return acc.reshape(B, T, H)
